# revision 26
# baseline (speedup 1.0000x reference)
"""Trainium2 Bass kernel for the circular drift-diffusion loss (batched expm).

Reference computes  loss = -mean_b log(relu(e_{idx_b}^T expm(t_b*A) p0_b) + eps)
with A a fixed 360x360 circular advection-diffusion operator, t_b in [0,1000),
p0_b a von Mises density, over a batch of 256.

Algorithm (per core; batch sharded 32/core over 8 cores):
  * Quantize t_b = m_b*T0 + r_b with T0 = 1000/2^K, m_b < 2^K.
  * M-chain: build M_j = expm(2^j*T0*A) by repeated squaring (prelude:
    ascending Taylor at T0, then K-2 squarings).  A squaring is 9 wide
    matmuls for S = M@M plus 9 PE transposes for S^T (the next stationary
    operand).
  * Q-chain (decoupled, lagging the M-chain off the critical path):
    p0 -> residual Taylor_DEG_R(r_b A) (Horner, host-precomputed r/k
    tables) -> masked batched matvecs Q <- bit_j ? M_j Q : Q.  The powers
    of A commute, so bits 0/1 lag two levels (Q warm-up), catch up at
    level 4, then lag one.  The top TWO bits reuse M_{K-2}: one apply +
    one double apply.
  * p0 on device (minimax poly cos + Exp activation); selection via
    one-hot mult + PE column-sum emits per-sample psel; the host does the
    O(B) log/mean epilogue (same class of glue as the host-side bit/mask
    prep).

Fast path (k_bits <= 10): all matrix/vector tiles bf16 -> PE matmuls run
1 cycle/row (vs 4 for f32), transposes 1 (vs 2), DVE gets 2-byte fast
modes, DMA halves.  PSUM stays f32; p0's phase pipeline stays f32.  Real
8-core hardware matches the numpy bit-simulation (loss rel-err 5.4e-3 on
the reference input; gate is 2e-2).  Heavy-diffusion inputs needing
deeper chains fall back to an f32 build of the same structure.

Scheduling notes (cost-model-driven): dependencies are tracked per
tile/PSUM-bank, so S and T live as per-row-chunk tiles and MT as three
PIECE-major tiles (MTp[i] = piece i of all rows = the stationary set of
the next level's matmul group i), each written by exactly one batched
evacuation; transposes stage a full trio into one PSUM bank and evacuate
once.  Inputs arrive packed in consumption order (chunk-0 pieces first)
so the first prelude matmuls start during the DMA phase.  Evacuations
are placed on DVE/ACT (gpsimd cannot touch PSUM) with the row gating the
next transposes split across both.  CopyPredicated masks are uint16
(hardware requires integer masks).
"""

import math

import numpy as np

# ---------------- static problem constants (hardcoded per contract) ----------
N = 360            # color mesh size
P = 120            # partition chunk (N = 3*P)
NCH = 3            # chunks
B = 256            # total batch
NCORES = 8
BL = B // NCORES   # per-core batch
QW = NCH * BL      # Q-chain tile width (96)
T_MAX = 1000.0
KAPPA = 400.0      # 1/SIGMA_INIT^2
EPS = 1e-5
TWO_PI = 6.283185307179586
# ln(1/(2*pi*i0e(400)))  [i0e(400) = 0.019953356281939987]
LNC = 2.076480848703078
# cos(sqrt(u)) on u in [0, pi^2] (|delta| folded to [0,pi]), minimax power
# basis c0..c5 (max err 1.75e-6 -> 7e-4 on log p0; tolerance is 2e-2)
COS_COEF = [0.9999982503105576, -0.4999925129381312, 0.0416590259231213,
            -0.0013857591185452258, 2.419643469550081e-05,
            -2.1969780329048054e-07]
# degree-8 Taylor-fit coefficients for the f32 fallback path (err 4e-14)
COS_COEF8 = [1.00000000e+00, -5.00000000e-01, 4.16666666e-02, -1.38888885e-03,
             2.48015646e-05, -2.75566515e-07, 2.08651966e-09, -1.13535474e-11,
             4.13131734e-14]

_COMPILED = {}

# fast-path dtype mode: "bf16" (2-byte tiles, PE 1cyc/row, DVE 2x modes) or
# "f32r" (f32 tiles, matmul operands viewed as float32r: PE 1cyc/row on wide
# matmuls at full f32 storage precision).  Chosen empirically on hardware.
FAST_MODE = "bf16"


def _taylor_deg(x, tol, lo):
    """Smallest d with x^(d+1)/(d+1)! < tol."""
    d = lo
    term = x ** (d + 1) / math.factorial(d + 1)
    while term > tol and d < 40:
        d += 1
        term *= x / (d + 1)
    return d


def _plan(anorm):
    """Choose (k_bits, deg_p, deg_r, mode) from ||A||_inf.  The time grid is
    T0 = T_MAX/2^k_bits; every squaring level applies one bit of the
    quantized delay."""
    xa = T_MAX * float(anorm)
    if xa <= 0.0:
        return 3, 4, 3, FAST_MODE

    def pick(c_lvl, c_pre, c_tay, tol_r, tol_p_num):
        k0 = max(3, min(16, math.ceil(math.log2(max(xa / 0.9, 2.0)))))
        best = None
        for k in range(max(3, k0 - 1), min(16, k0 + 2) + 1):
            x0 = xa / (1 << k)
            tol_p = min(max(tol_p_num / 2 ** (k / 2), 5e-8), 2e-5)
            dp = _taylor_deg(x0, tol_p, 4)
            dr = _taylor_deg(x0, tol_r, 3)
            cost = (k - 1) * c_lvl + (dp - 1) * c_pre + dr * c_tay
            if best is None or cost < best[0]:
                best = (cost, k, dp, dr)
        return best[1], best[2], best[3]

    fast_c = {"bf16": (2.2, 1.6, 0.6), "f32r": (2.5, 1.7, 0.7)}[FAST_MODE]
    k, dp, dr = pick(*fast_c, 2e-4, 3e-4)
    if k <= 10:
        return k, dp, dr, FAST_MODE
    k, dp, dr = pick(7.6, 4.5, 0.7, 1e-6, 3e-5)
    return k, dp, dr, "f32"


def _build_bass(k_bits, deg_p, deg_r, mode):
    """Construct the Bass program (SPMD; identical on all 8 cores)."""
    import concourse.tile as tile
    from concourse import bacc, mybir

    F32 = mybir.dt.float32
    R32 = mybir.dt.float32r
    BF = mybir.dt.bfloat16 if mode == "bf16" else F32
    # mask dtype: CopyPredicated requires an integer mask on hardware;
    # uint16 keeps the 2-byte element size in bf16 mode
    MDT = mybir.dt.uint16 if mode == "bf16" else mybir.dt.uint8
    AF = mybir.ActivationFunctionType
    OP = mybir.AluOpType
    cos_coef = COS_COEF if mode != "f32" else COS_COEF8

    def mv(ap):
        # matmul operand view: f32r reinterpretation in f32r mode
        return ap.bitcast(R32) if mode == "f32r" else ap

    nc = bacc.Bacc("TRN2", target_bir_lowering=False, debug=False)

    def din(name, shape, dt=F32):
        return nc.dram_tensor(name, shape, dt, kind="ExternalInput").ap()

    d_xx = din("xx", [P, 5 * N + 3 * P], BF)   # packed X / X^T pieces
    d_cmir = din("cmir", [P, NCH + BL])        # [c_mesh chunks | init rep]
    d_qtab = din("qtab", [P, (deg_r + 1) * QW], BF)  # [rdk | one-hot]
    d_msk = din("msk", [P, k_bits * QW], MDT)  # bit masks (0/1) x3 chunks
    d_out = nc.dram_tensor("psel", [1, BL], F32,
                           kind="ExternalOutput").ap()

    with tile.TileContext(nc) as tc:
        with (
            tc.tile_pool(name="const", bufs=1) as cpool,
            tc.tile_pool(name="mats", bufs=4) as mpool,
            tc.tile_pool(name="qp", bufs=3) as qpool,
            tc.tile_pool(name="tp", bufs=4) as tpool,
            tc.tile_pool(name="psb", bufs=3, space="PSUM") as psb,
            tc.tile_pool(name="pst", bufs=3, space="PSUM") as pstp,
            tc.tile_pool(name="pss", bufs=2, space="PSUM") as pss,
        ):
            # ---- input DMAs: few, packed, in consumption order ------------
            # xx layout: [XT00 | XNr0 | XTr1 | XNr1 | XTr2 | XNr2 | XT0rest]
            # so each DMA lands exactly what the next prelude matmuls need.
            XXW = 5 * N + 3 * P
            cuts = [0, P + N, P + 3 * N, P + 5 * N, XXW]
            XXT = []
            for j, (a, b) in enumerate(zip(cuts[:-1], cuts[1:])):
                xx_j = cpool.tile([P, b - a], BF, tag=f"xx{j}")
                XXT.append(xx_j)
            CMIR = cpool.tile([P, NCH + BL], F32, tag="cmir")
            QTAB = cpool.tile([P, (deg_r + 1) * QW], BF, tag="qtab")
            MSK = cpool.tile([P, k_bits * QW], MDT, tag="msk")
            for j, (a, b) in enumerate(zip(cuts[:-1], cuts[1:])):
                nc.sync.dma_start(XXT[j][:], d_xx[:, a:b])
            nc.scalar.dma_start(CMIR[:], d_cmir[:])
            nc.sync.dma_start(QTAB[:], d_qtab[:])
            nc.sync.dma_start(MSK[:], d_msk[:])
            CM = CMIR[:, 0:NCH]
            IREP = CMIR[:, NCH:NCH + BL]
            RDK = QTAB[:, 0:deg_r * QW]
            OH = QTAB[:, deg_r * QW:(deg_r + 1) * QW]

            def _xx(off, w):
                for j, (a, b) in enumerate(zip(cuts[:-1], cuts[1:])):
                    if a <= off and off + w <= b:
                        return XXT[j][:, off - a:off - a + w]
                raise AssertionError("xx slice crosses DMA boundary")

            def xn_s(c):
                # XN row-chunk c (rhs of prelude matmuls), contiguous
                return _xx(P + 2 * c * N, N)

            def xt_s(c, i):
                # XT block (row-chunk c, piece i) for prelude/taylor lhsT
                if c == 0:
                    o = 0 if i == 0 else P + 5 * N + (i - 1) * P
                else:
                    o = P + (2 * c - 1) * N + i * P
                return _xx(o, P)

            ONES = cpool.tile([P, 1], BF, tag="ones")
            nc.vector.memset(ONES[:], 1.0)
            BEXP = cpool.tile([P, 1], F32, tag="bexp")
            nc.vector.memset(BEXP[:], LNC - KAPPA)

            # identities built on device (no DMA)
            EYE = cpool.tile([P, NCH * N], BF, tag="eye")
            nc.vector.memset(EYE[:], 1.0)
            nc.gpsimd.affine_select(
                EYE[:].rearrange("p (c n) -> p c n", c=NCH),
                EYE[:].rearrange("p (c n) -> p c n", c=NCH),
                pattern=[[-P, NCH], [1, N]], compare_op=OP.is_equal,
                fill=0.0, base=0, channel_multiplier=-1,
            )
            E120 = cpool.tile([P, P], BF, tag="e120")
            nc.vector.memset(E120[:], 1.0)
            nc.gpsimd.affine_select(
                E120[:], E120[:], pattern=[[1, P]], compare_op=OP.is_equal,
                fill=0.0, base=0, channel_multiplier=-1,
            )

            def mm_group(ps, lhsT_of, rhs_of, i):
                # lhsT_of(c, i) -> [P,P] slice; rhs_of(c) -> [P,w] slice
                for c in range(NCH):
                    nc.tensor.matmul(
                        ps[:], lhsT=mv(lhsT_of(c, i)), rhs=mv(rhs_of(c)),
                        start=(c == 0), stop=(c == NCH - 1),
                    )

            def tile_b(tile_, c, i):
                return tile_[:, c * N + i * P: c * N + i * P + P]

            # ---- p0 (von Mises) in Q-layout [P, 3*BL] ---------------------
            Q0 = qpool.tile([P, QW], BF, tag="q")
            deg_c = len(cos_coef) - 1
            for c in range(NCH):
                qs = Q0[:, c * BL:(c + 1) * BL]
                dl = tpool.tile([P, BL], F32, tag="t0")
                # delta = init - c_mesh  (cos is even, sign irrelevant)
                nc.vector.tensor_scalar(dl[:], IREP[:], CM[:, c:c + 1], None,
                                        op0=OP.subtract)
                ab = tpool.tile([P, BL], F32, tag="t1")
                nc.scalar.activation(ab[:], dl[:], AF.Abs)
                fl = tpool.tile([P, BL], F32, tag="t2")
                nc.vector.tensor_scalar(fl[:], ab[:], -1.0, TWO_PI,
                                        op0=OP.mult, op1=OP.add)
                w = tpool.tile([P, BL], F32, tag="t3")
                nc.vector.tensor_tensor(w[:], ab[:], fl[:], op=OP.min)
                u = tpool.tile([P, BL], F32, tag="t0")
                nc.vector.tensor_tensor(u[:], w[:], w[:], op=OP.mult)
                h = tpool.tile([P, BL], F32, tag="t1")
                nc.vector.tensor_scalar(h[:], u[:], cos_coef[deg_c],
                                        cos_coef[deg_c - 1],
                                        op0=OP.mult, op1=OP.add)
                heng = nc.gpsimd if c == 1 else nc.vector
                for k in range(deg_c - 2, -1, -1):
                    heng.tensor_tensor(h[:], h[:], u[:], op=OP.mult)
                    heng.tensor_scalar(h[:], h[:], cos_coef[k], None,
                                       op0=OP.add)
                # p0 = exp(kappa*cos - kappa + lnC)
                nc.scalar.activation(qs, h[:], AF.Exp, bias=BEXP[:],
                                     scale=KAPPA)

            # ---- Q-chain step emitters (off the M-chain critical path) ----
            # residual Taylor: V <- Q0 + rdk_k*(X V), k=deg_r..1
            taylor_state = {"V": Q0, "k": deg_r}

            def taylor_step():
                k = taylor_state["k"]
                if k < 1:
                    return
                Vc = taylor_state["V"]
                ps = pss.tile([P, QW], F32, tag="ap")
                for i in range(NCH):
                    for c in range(NCH):
                        nc.tensor.matmul(
                            ps[:, i * BL:(i + 1) * BL],
                            lhsT=mv(xt_s(c, i)),
                            rhs=mv(Vc[:, c * BL:(c + 1) * BL]),
                            start=(c == 0), stop=(c == NCH - 1),
                        )
                Vn = qpool.tile([P, QW], BF, tag="v")
                nc.vector.tensor_tensor(Vn[:], ps[:],
                                        RDK[:, (k - 1) * QW: k * QW],
                                        op=OP.mult)
                nc.gpsimd.tensor_tensor(Vn[:], Vn[:], Q0[:], op=OP.add)
                taylor_state["V"] = Vn
                taylor_state["k"] = k - 1

            def apply_bit(lhsT_of, q_tile, bit, blend_src=None):
                # Qn = bit ? M_j @ q : blend_src   (9 narrow mms + one blend)
                # the pass-through copy goes first: it only needs q, so it
                # overlaps the matmuls instead of serializing after them
                Qn = qpool.tile([P, QW], BF, tag="q")
                nc.gpsimd.tensor_copy(
                    Qn[:], (q_tile if blend_src is None else blend_src)[:])
                ps = pss.tile([P, QW], F32, tag="ap")
                for i in range(NCH):
                    for c in range(NCH):
                        nc.tensor.matmul(
                            ps[:, i * BL:(i + 1) * BL],
                            lhsT=mv(lhsT_of(c, i)),
                            rhs=mv(q_tile[:, c * BL:(c + 1) * BL]),
                            start=(c == 0), stop=(c == NCH - 1),
                        )
                nc.vector.copy_predicated(
                    Qn[:], MSK[:, bit * QW:(bit + 1) * QW], ps[:])
                return Qn

            # ---- prelude: ascending Taylor S = I + sum X^k/k! -------------
            # S and T live as three per-row-chunk [P,N] tiles so every
            # evacuation is a whole-tile write: the dependency tracker is
            # tile/bank-granular, and single-writer tiles keep consumers
            # from waiting on unrelated evacuations.
            Srows = []
            for c in range(NCH):
                s_c = mpool.tile([P, N], BF, tag=f"S{c}")
                nc.vector.tensor_tensor(s_c[:], xn_s(c),
                                        EYE[:, c * N:(c + 1) * N], op=OP.add)
                Srows.append(s_c)
            Trows = None
            for k in range(2, deg_p + 1):
                Tn = []
                for i in range(NCH):
                    t_i = mpool.tile([P, N], BF, tag=f"T{i}")
                    Tn.append(t_i)
                if Trows is None:
                    rhs_of = xn_s
                else:
                    rhs_of = (lambda c, _T=Trows: _T[c][:])
                for i in range(NCH):
                    ps = psb.tile([P, N], F32, tag="sq")
                    mm_group(ps, xt_s, rhs_of, i)
                    # scale-evac feeds the next step's matmuls; the S
                    # accumulation runs on DVE off the PE path
                    if i == 1:
                        nc.vector.tensor_scalar(Tn[i][:], ps[:], 1.0 / k,
                                                None, op0=OP.mult)
                    else:
                        nc.scalar.mul(Tn[i][:], ps[:], 1.0 / k)
                    aeng = nc.vector if k == deg_p else nc.gpsimd
                    aeng.tensor_tensor(Srows[i][:], Srows[i][:],
                                       Tn[i][:], op=OP.add)
                Trows = Tn
                if k < deg_p:
                    # the slot after the last step would block the PE right
                    # before the first transposes; drain later instead
                    taylor_step()

            # MT lives as three PIECE-major tiles: MTp[i] holds piece i of
            # all three MT rows, i.e. exactly the stationary set the next
            # level's matmul group i consumes -- one trio of transposes
            # fills one PSUM bank, one [P,N] copy fills one tile.
            def transpose_trio(MTpn, Sr, ib, eng):
                pt = pstp.tile([P, N], BF, tag="tr")
                for cp in range(NCH):
                    nc.tensor.transpose(
                        mv(pt[:, cp * P:(cp + 1) * P]),
                        mv(Sr[ib][:, cp * P:(cp + 1) * P]),
                        mv(E120[:]),
                    )
                if eng == "split":
                    h = N // 2
                    nc.vector.tensor_copy(MTpn[ib][:, 0:h], pt[:, 0:h])
                    nc.scalar.copy(MTpn[ib][:, h:N], pt[:, h:N])
                elif eng is nc.vector:
                    nc.vector.tensor_copy(MTpn[ib][:], pt[:])
                elif eng is nc.scalar:
                    nc.scalar.copy(MTpn[ib][:], pt[:])
                else:
                    nc.gpsimd.tensor_copy(MTpn[ib][:], pt[:])

            def new_mtp():
                out = []
                for i in range(NCH):
                    mtp_i = mpool.tile([P, N], BF, tag=f"MTp{i}")
                    out.append(mtp_i)
                return out

            def mtp_acc(MTp):
                return lambda c, i: MTp[i][:, c * P:(c + 1) * P]

            tr_engs = ("split", nc.vector, nc.scalar)
            mtps = [new_mtp()]
            for ib in range(NCH):
                transpose_trio(mtps[0], Srows, ib, tr_engs[ib])
            while taylor_state["k"] >= 1:   # drain (off the M-chain path)
                taylor_step()
            QB = taylor_state["V"]

            # ---- chain: squarings, with commuting bit applies lagged two
            # levels so the Q-chain (p0 -> residual Taylor) has time to
            # finish off the critical path.  All M_j share eigenvectors, so
            # bit applies can run in any order.
            n_sq = k_bits - 2

            def _sched_level(b):
                # bits 0/1 lag two levels (Q-chain warm-up), bits 2/3 both
                # land at level 4, later bits lag one level
                return min(b + 2, max(b + 1, 4))

            sched = {}
            for b in range(k_bits - 2):
                s_b = _sched_level(b)
                if s_b <= n_sq:
                    sched.setdefault(s_b, []).append(b)
            for s in range(1, n_sq + 1):
                Sn = []
                for i in range(NCH):
                    sn_i = mpool.tile([P, N], BF, tag=f"S{i}")
                    Sn.append(sn_i)
                for i in range(NCH):
                    ps = psb.tile([P, N], F32, tag="sq")
                    mm_group(ps, mtp_acc(mtps[s - 1]),
                             lambda c, _S=Srows: _S[c][:], i)
                    h = N // 2
                    if i == 0:
                        nc.vector.tensor_copy(Sn[i][:], ps[:])
                    elif i == 1:
                        nc.scalar.copy(Sn[i][:], ps[:])
                    else:
                        # the last row gates this level's transposes: split
                        # it across DVE+ACT so it lands fastest
                        nc.vector.tensor_copy(Sn[i][:, 0:h], ps[:, 0:h])
                        nc.scalar.copy(Sn[i][:, h:N], ps[:, h:N])
                MTpn = new_mtp()
                transpose_trio(MTpn, Sn, 0, tr_engs[0])
                transpose_trio(MTpn, Sn, 1, tr_engs[1])
                transpose_trio(MTpn, Sn, 2, tr_engs[2])
                for b in sched.get(s, []):
                    QB = apply_bit(mtp_acc(mtps[b]), QB, b)
                mtps.append(MTpn)
                Srows = Sn

            # ---- remaining low bits (small-k stragglers) ------------------
            for b in range(k_bits - 2):
                if _sched_level(b) > n_sq:
                    QB = apply_bit(mtp_acc(mtps[b]), QB, b)

            # ---- top two bits: single + double apply of M_{k-2} -----------
            top = mtp_acc(mtps[n_sq])
            QB = apply_bit(top, QB, k_bits - 2)
            psy = pss.tile([P, QW], F32, tag="ap")
            for i in range(NCH):
                for c in range(NCH):
                    nc.tensor.matmul(
                        psy[:, i * BL:(i + 1) * BL],
                        lhsT=mv(top(c, i)),
                        rhs=mv(QB[:, c * BL:(c + 1) * BL]),
                        start=(c == 0), stop=(c == NCH - 1),
                    )
            Y1 = qpool.tile([P, QW], BF, tag="v")
            hq = QW // 2
            nc.vector.tensor_copy(Y1[:, 0:hq], psy[:, 0:hq])
            nc.scalar.copy(Y1[:, hq:QW], psy[:, hq:QW])
            Vf = apply_bit(top, Y1, k_bits - 1, blend_src=QB)

            # ---- selection: psel_b = e_idx^T V (one-hot dot); the
            # log/mean epilogue runs on host over the 256 gathered values
            sel = psb.tile([1, BL], F32, tag="sq")
            tmp = tpool.tile([P, QW], BF, tag="sel")
            nc.vector.tensor_tensor(tmp[:], Vf[:], OH[:], op=OP.mult)
            for c in range(NCH):
                nc.tensor.matmul(sel[:], lhsT=mv(ONES[:]),
                                 rhs=mv(tmp[:, c * BL:(c + 1) * BL]),
                                 start=(c == 0), stop=(c == NCH - 1))
            out = tpool.tile([1, BL], F32, tag="r0")
            nc.vector.tensor_copy(out[:], sel[:])
            nc.sync.dma_start(d_out[:], out[:])

    nc.compile()
    return nc


def _host_prep(c_mesh, gtheta, sigma_diff, init_color, delay_t, report_color):
    """Host-side glue: operator assembly (replicating reference f32 ops),
    plan selection, and per-core index/bit/layout arrays."""
    import ml_dtypes
    f32 = np.float32
    c = np.asarray(c_mesh, dtype=f32)
    g = np.asarray(gtheta, dtype=f32)
    s = np.asarray(sigma_diff, dtype=f32)[0]
    init = np.asarray(init_color, dtype=f32)
    t = np.asarray(delay_t, dtype=f32)
    rep = np.asarray(report_color, dtype=f32)

    d = (c[1] - c[0]).astype(f32)
    eye = np.eye(N, dtype=f32)
    up = np.roll(eye, -1, axis=1)
    dn = np.roll(eye, 1, axis=1)
    D1 = ((up - dn) / (f32(2.0) * d)).astype(f32)
    D2 = ((up - f32(2.0) * eye + dn) / (d * d)).astype(f32)
    A = ((s ** f32(2.0)) / f32(2.0) * D2 - D1 * g[None, :]).astype(f32)

    anorm = np.abs(A.astype(np.float64)).sum(axis=1).max()
    k_bits, deg_p, deg_r, mode = plan = _plan(anorm)
    bf = ml_dtypes.bfloat16 if mode == "bf16" else f32
    mdt = np.uint16 if mode == "bf16" else np.uint8
    T0 = T_MAX / (1 << k_bits)
    X = (A * f32(T0)).astype(f32)

    m = np.floor(t.astype(np.float64) / T0).astype(np.int64)
    m = np.clip(m, 0, (1 << k_bits) - 1)
    r = (t.astype(np.float64) - m * T0) / T0  # in X = T0*A units
    bits = ((m[:, None] >> np.arange(k_bits)[None, :]) & 1)     # [B, K]
    idx = np.argmin(np.abs(c[None, :] - rep[:, None]), axis=1)

    # packed matrix buffer in DMA/consumption order:
    # [XT00 | XNr0 | XTr1 | XNr1 | XTr2 | XNr2 | XT0rest]
    XT_ = np.ascontiguousarray(X.T)
    xx = np.concatenate([
        XT_[0:P, 0:P],
        X[0:P, :], XT_[P:2 * P, :],
        X[P:2 * P, :], XT_[2 * P:3 * P, :],
        X[2 * P:3 * P, :], XT_[0:P, P:N],
    ], axis=1)
    cm = np.ascontiguousarray(c.reshape(NCH, P).T)
    shared = {"xx": xx.astype(bf)}
    in_maps = []
    for core in range(NCORES):
        sl = slice(core * BL, (core + 1) * BL)
        irep = np.broadcast_to(init[sl][None, :], (P, BL)).astype(f32)
        cmir = np.concatenate([cm, irep], axis=1).astype(f32)
        # bit j replicated x3 (one copy per Q chunk) at [j*QW:(j+1)*QW]
        msk = np.broadcast_to(
            np.tile(bits[sl].T.reshape(k_bits, 1, BL), (1, NCH, 1))
            .reshape(1, k_bits * QW), (P, k_bits * QW)).astype(mdt)
        rdk = np.empty((deg_r, BL), f32)
        for k in range(1, deg_r + 1):
            rdk[k - 1] = (r[sl] / k).astype(f32)
        rdk = np.tile(rdk.reshape(deg_r, 1, BL), (1, NCH, 1)).reshape(
            1, deg_r * QW)
        oh = np.zeros((NCH, P, BL), f32)
        for b, ix in enumerate(idx[sl]):
            oh[ix // P, ix % P, b] = 1.0
        oh = oh.transpose(1, 0, 2).reshape(P, QW)
        qtab = np.concatenate(
            [np.broadcast_to(rdk, (P, deg_r * QW)), oh], axis=1).astype(bf)
        in_maps.append(dict(shared, cmir=cmir, msk=np.ascontiguousarray(msk),
                            qtab=np.ascontiguousarray(qtab)))
    return plan, in_maps


def _get_nc(plan):
    if plan not in _COMPILED:
        _COMPILED[plan] = _build_bass(*plan)
    return _COMPILED[plan]


def kernel(**inputs):
    from concourse.bass_utils import run_bass_kernel_spmd

    plan, in_maps = _host_prep(
        inputs["c_mesh"], inputs["gtheta"], inputs["sigma_diff"],
        inputs["init_color"], inputs["delay_t"], inputs["report_color"],
    )
    nc = _get_nc(plan)
    res = run_bass_kernel_spmd(nc, in_maps, list(range(NCORES)))
    psel = np.concatenate(
        [np.asarray(res.results[k]["psel"]).reshape(-1)
         for k in range(NCORES)])
    terms = np.log(np.maximum(psel.astype(np.float64), 0.0) + EPS)
    loss = -np.mean(terms)
    return np.asarray(loss, dtype=np.float32)
